# revision 21
# baseline (speedup 1.0000x reference)
"""Trainium2 Bass kernel for MeshGNN message passing (8 NeuronCores, SPMD).

Math reformulation (exact): since softmax weights sum to 1 and the output MLP is
linear, fold W_concat/W_out into per-node quantities:
    M1 = W_out @ W_concat[:, :128]   [3,128]
    M2 = W_out @ W_concat[:, 128:]   [3,3]
    c0 = b_concat @ W_out.T + b_out  [3]
    kx[j] = x[j] @ W_k.T + b_k                  (64,)   -> table
    w[j]  = x[j] @ M1.T + p[j] @ M2.T           (3,)    -> table
    q[n]  = (x[n] @ W_q.T + b_q) / scale        (64,)
    scores[n,k] = q[n] . kx[nbr]
    e = exp(scores * (nbr != 0))                         (scores bounded ~+-3)
    out[n] = p[n] + (-v[n] + c0) + sum_k e_k * w[nbr] / sum_k e_k,  v = p @ M2.T

Implementation notes (v5):
- Table rows are 128B: kx in fp8 e4m3 (64B) + w in fp16 (6B) + pad. The
  gather phase is DMA-transfer-bound (~35ns per random 512B descriptor on
  HW), so halving the row size halves the dominant cost. b_k is dropped from
  kx (softmax shift-invariance makes it a no-op except for the ~0.02%
  padding-masked edges, ~3e-4 rel err).
- Rows are stored in DRAM in (core, partition, tile) order so the SBUF->DRAM
  staging write is contiguous per partition (128 big descriptors instead of
  7552 strided ones, which cost ~90us of HWDGE desc-gen in v1).
- Rows are fetched with dma_gather in PAIRS (256B, idx = row//2 fits int16)
  and the correct half is selected arithmetically via host-prepared parity
  masks. Host computes pair/parity from the permuted row position.
- dma_gather desc-gen runs ~30us per 3840-idx chunk on a Q7 cpu pair, but the
  4 SWDGE queues desc-gen CONCURRENTLY when gathers are issued non-prepared
  (prepare_only blocks the Pool engine per prep and serializes the cluster,
  measured 3.4x slower end-to-end - do not use it here).
- Per-chunk DVE work is only 4 ops (score products + reduce, w select +
  reduce); softmax/value epilogue is batched once over all 59 tiles.
"""

import sys

import numpy as np

sys.path.insert(0, "/opt/trn_rl_repo")

import concourse.bass as bass
import concourse.mybir as mybir
import concourse.tile as tile
from concourse import bacc
from concourse.bass import ds, ts
from concourse.bass_utils import run_bass_kernel_spmd

N_CORES = 8
H = 128
K = 15
DT = mybir.dt
F16 = DT.float16
F32 = DT.float32
I16 = DT.int16

F8 = DT.float8e4

ROW = 128                 # fp8 elems per table row (128B): kx f8[64]|w f16[3]|pad
PAIR = 2 * ROW            # gather element: two rows (256B)
QC = 64                   # q/k dim
W1C = 131                 # matmul cols: kx(64)|w(3)|q(64)
NQ = 4                    # SWDGE queues
GATHER_BUFS = 8


def build_program(n_total, shard, n_tiles, chunk_tiles=2):
    P = 128
    nc = bacc.Bacc(None, debug=False, num_swdge_queues=NQ)

    ax = nc.declare_dram_parameter("ax", [P, shard], F16, isOutput=False)    # x.T
    pts = nc.declare_dram_parameter("pts", [P, n_tiles * 3], F32, isOutput=False)
    idx16 = nc.declare_dram_parameter("idx16", [P, n_tiles * K * 8], I16,
                                      isOutput=False)
    cmask = nc.declare_dram_parameter("cmask", [P, n_tiles * K * 2], F16,
                                      isOutput=False)
    pmask = nc.declare_dram_parameter("pmask", [P, n_tiles * K * 2], F16,
                                      isOutput=False)
    w1 = nc.declare_dram_parameter("w1", [P, W1C], F16, isOutput=False)
    brow = nc.declare_dram_parameter("brow", [P, W1C], F16, isOutput=False)
    c0r = nc.declare_dram_parameter("c0r", [P, 3], F32, isOutput=False)
    m2r = nc.declare_dram_parameter("m2r", [P, 9], F32, isOutput=False)
    out = nc.declare_dram_parameter("out", [P, n_tiles * 3], F32, isOutput=True)

    NT = n_tiles
    with tile.TileContext(nc) as tc:
        with (
            tc.tile_pool(name="persist", bufs=1) as pp,
            tc.tile_pool(name="dram", bufs=1, space="DRAM") as dp,
            tc.tile_pool(name="psum", bufs=4, space="PSUM") as psp,
            tc.tile_pool(name="kxgp", bufs=GATHER_BUFS) as kxgp,
            # bufs=2 so the ACT fp8->fp16 convert of chunk i+1 overlaps the
            # DVE score math of chunk i (bufs=1 ping-pongs the chain serially)
            tc.tile_pool(name="work", bufs=2) as wp,
        ):
            # ---- persistent SBUF ----
            xT = pp.tile([P, shard], F16)
            pts_sb = pp.tile([P, NT * 3], F32)
            idx_sb = pp.tile([P, NT * K * 8], I16)
            cm_sb = pp.tile([P, NT * K * 2], F16)
            pm_sb = pp.tile([P, NT * K * 2], F16)
            w1_sb = pp.tile([P, W1C], F16)
            br_sb = pp.tile([P, W1C], F16)
            c0_sb = pp.tile([P, 3], F32)
            m2_sb = pp.tile([P, 9], F32)
            q_sb = pp.tile([P, NT * QC], F16)
            base_sb = pp.tile([P, NT * 3], F32)
            out_sb = pp.tile([P, NT * 3], F32)
            tblall_sb = pp.tile([P, NT * ROW], F8)
            s2a_all = pp.tile([P, NT * K * 2], F16)
            wsel_all = pp.tile([P, NT * K * 3], F16)

            table_pad = dp.tile([shard, ROW], F8, space="DRAM")
            table_full = dp.tile([N_CORES * shard, ROW], F8, space="DRAM",
                                 addr_space="Shared")
            bar_in = dp.tile([16, 4], F32, space="DRAM")
            bar_out = dp.tile([128, 4], F32, space="DRAM")

            # dummy barrier collective, issued before everything else: the
            # CC-core state machine (~26us) and the cross-core rendezvous
            # (~17us) run during the input loads instead of delaying the
            # real AllGather (values are never read)
            nc.gpsimd.collective_compute(
                "AllGather",
                mybir.AluOpType.bypass,
                replica_groups=[list(range(N_CORES))],
                ins=[bar_in[:].opt()],
                outs=[bar_out[:].opt()],
            )

            # xT + weights first: phase 1 (and thus the collective) starts as
            # early as possible; masks/indices are not needed until the gather
            # phase (~120us in)
            nc.sync.dma_start(out=w1_sb[:], in_=w1[:, :])
            nc.sync.dma_start(out=xT[:], in_=ax[:, :])
            nc.sync.dma_start(out=pts_sb[:], in_=pts[:, :])
            nc.sync.dma_start(out=br_sb[:], in_=brow[:, :])
            nc.sync.dma_start(out=c0_sb[:], in_=c0r[:, :])
            nc.sync.dma_start(out=m2_sb[:], in_=m2r[:, :])
            nc.sync.dma_start(out=idx_sb[:], in_=idx16[:, :])
            nc.sync.dma_start(out=cm_sb[:], in_=cmask[:, :])
            nc.sync.dma_start(out=pm_sb[:], in_=pmask[:, :])

            table_pairs = table_full[:].rearrange("(a two) e -> a (two e)", two=2)
            # small chunks at the START (the Q7 cluster serializes the first
            # few desc-gens while the pipeline fills) and at the END (short
            # drain: desc-gen + DMA + convert + DVE chain of the last chunk)
            chunks = []
            t0 = 0
            for _ in range(6):
                chunks.append((t0, 1))
                t0 += 1
            while t0 < n_tiles - 3:
                chunks.append((t0, min(chunk_tiles, n_tiles - 3 - t0)))
                t0 += chunk_tiles
            while t0 < n_tiles:
                chunks.append((t0, 1))
                t0 += 1

            def emit_gather(ci):
                t0c, nt = chunks[ci]
                nidx = nt * K * P
                kxg = kxgp.tile([P, chunk_tiles * K * PAIR], F8, tag="kxg")
                nc.gpsimd.dma_gather(
                    kxg[:, 0:nt * K * PAIR].rearrange("p (s e) -> p s e", e=PAIR),
                    table_pairs,
                    idx_sb[:, ds(t0c * K * 8, nt * K * 8)],
                    nidx, nidx, PAIR,
                    single_packet=False,
                    queue_num=ci % NQ,
                )
                return kxg

            # ---- phase 1: matmuls -> PSUM -> (table rows, q) via ACT ----
            # table row (128B): kx fp8 at byte [0:64), w fp16 at bytes [64:70).
            # b_k is dropped from kx: q.b_k is constant per node, and softmax
            # is shift-invariant (the ~0.02% padding-masked edges see a tiny
            # shift error ~3e-4 rel, well inside budget).
            tbl3 = tblall_sb[:].rearrange("p (t e) -> p t e", e=ROW)
            tblw = tbl3[:, :, 64:70].bitcast(F16)           # [P, NT, 3] f16
            q3 = q_sb[:].rearrange("p (t e) -> p t e", e=QC)
            t0 = 0
            while t0 < n_tiles:
                g = min(3, n_tiles - t0)
                ps = psp.tile([P, 3 * W1C], F32, space="PSUM", tag="ps")
                for j in range(g):
                    nc.tensor.matmul(out=ps[:, ts(j, W1C)],
                                     lhsT=xT[:, ts(t0 + j, P)], rhs=w1_sb[:],
                                     start=True, stop=True)
                ps3 = ps[:, 0:g * W1C].rearrange("p (t c) -> p t c", c=W1C)
                nc.scalar.copy(out=tbl3[:, t0:t0 + g, 0:64],
                               in_=ps3[:, :, 0:64])
                nc.scalar.copy(out=tblw[:, t0:t0 + g, :], in_=ps3[:, :, 64:67])
                nc.scalar.copy(out=q3[:, t0:t0 + g, :], in_=ps3[:, :, 67:W1C])
                t0 += 3

            # v = p @ M2.T for all tiles: [P, NT, 3]
            vp_all = pp.tile([P, NT * 9], F32)
            for j in range(3):
                nc.vector.tensor_tensor(
                    out=vp_all[:].rearrange("p (t j i) -> p t j i", j=3, i=3)
                        [:, :, j, :],
                    in0=pts_sb[:].rearrange("p (t i) -> p t i", i=3),
                    in1=m2_sb[:, ds(3 * j, 3)].unsqueeze(1)
                        .broadcast_to([P, NT, 3]),
                    op=mybir.AluOpType.mult)
            v_all = pp.tile([P, NT * 3], F32)
            nc.vector.tensor_reduce(
                out=v_all[:],
                in_=vp_all[:].rearrange("p (t j i) -> p (t j) i", j=3, i=3),
                axis=mybir.AxisListType.X, op=mybir.AluOpType.add)
            v16 = pp.tile([P, NT * 3], F16)
            nc.vector.tensor_copy(out=v16[:], in_=v_all[:])
            nc.vector.tensor_tensor(
                out=tblw, in0=tblw,
                in1=v16[:].rearrange("p (t i) -> p t i", i=3),
                op=mybir.AluOpType.add)
            nc.vector.tensor_tensor(
                out=q3, in0=q3,
                in1=br_sb[:, 67:W1C].unsqueeze(1).broadcast_to([P, NT, QC]),
                op=mybir.AluOpType.add)
            b1_all = pp.tile([P, NT * 3], F32)
            nc.vector.tensor_tensor(
                out=b1_all[:].rearrange("p (t i) -> p t i", i=3),
                in0=c0_sb[:].unsqueeze(1).broadcast_to([P, NT, 3]),
                in1=v_all[:].rearrange("p (t i) -> p t i", i=3),
                op=mybir.AluOpType.subtract)
            nc.vector.tensor_tensor(
                out=base_sb[:], in0=b1_all[:], in1=pts_sb[:],
                op=mybir.AluOpType.add)

            # ---- contiguous table write: row order (p, t) per core ----
            nc.sync.dma_start(
                out=table_pad[:].rearrange("(p t) e -> p (t e)", p=P),
                in_=tblall_sb[:])

            # ---- all-gather the fp16 table ----
            nc.gpsimd.collective_compute(
                "AllGather",
                mybir.AluOpType.bypass,
                replica_groups=[list(range(N_CORES))],
                ins=[table_pad[:].opt()],
                outs=[table_full[:].opt()],
            )

            # ---- per-chunk: trigger gather + score/value partials ----
            def emit_compute(ci, kxg):
                t0c, nt = chunks[ci]
                kx4 = kxg[:, 0:nt * K * PAIR].rearrange(
                    "p (t s e) -> p t s e", s=2 * K, e=ROW)
                # fp8 kx -> fp16 on the (otherwise idle) ACT engine
                kxf = wp.tile([P, chunk_tiles * K * 2 * QC], F16, tag="kxf")
                kxf4 = kxf[:, 0:nt * K * 2 * QC].rearrange(
                    "p (t s e) -> p t s e", s=2 * K, e=QC)
                nc.scalar.copy(out=kxf4, in_=kx4[:, :, :, 0:QC])
                qc_ap = (q_sb[:, ds(t0c * QC, nt * QC)]
                         .rearrange("p (t e) -> p t e", e=QC)
                         .unsqueeze(2).broadcast_to([P, nt, 2 * K, QC]))
                prod = wp.tile([P, chunk_tiles * K * 2 * QC], F16, tag="prod")
                pr5 = prod[:, 0:nt * K * 2 * QC].rearrange(
                    "p (t s e) -> p t s e", s=2 * K, e=QC)
                nc.vector.tensor_tensor(out=pr5, in0=kxf4,
                                        in1=qc_ap, op=mybir.AluOpType.mult)
                nc.vector.tensor_reduce(
                    out=s2a_all[:, ds(t0c * K * 2, nt * K * 2)], in_=pr5,
                    axis=mybir.AxisListType.X, op=mybir.AluOpType.add)
                wstage = wp.tile([P, chunk_tiles * K * 2 * 3], F16, tag="wst")
                ws4 = wstage[:, 0:nt * K * 2 * 3].rearrange(
                    "p (t s e) -> p t s e", s=2 * K, e=3)
                nc.vector.tensor_tensor(
                    out=ws4, in0=kx4[:, :, :, 64:70].bitcast(F16),
                    in1=pm_sb[:, ds(t0c * K * 2, nt * K * 2)]
                        .rearrange("p (t s) -> p t s", s=2 * K)
                        .unsqueeze(3).broadcast_to([P, nt, 2 * K, 3]),
                    op=mybir.AluOpType.mult)
                nc.vector.tensor_reduce(
                    out=wsel_all[:, ds(t0c * K * 3, nt * K * 3)]
                        .rearrange("p (tk e) -> p tk e", e=3),
                    in_=wstage[:, 0:nt * K * 2 * 3].rearrange(
                        "p (tk h e) -> p tk e h", h=2, e=3),
                    axis=mybir.AxisListType.X, op=mybir.AluOpType.add)

            with nc.allow_low_precision(
                    reason="fp16 attention intermediates; rel-err budget 2e-2"):
                for ci in range(len(chunks)):
                    emit_compute(ci, emit_gather(ci))

                # ---- batched softmax / value epilogue over all tiles ----
                sm2a = pp.tile([P, NT * K * 2], F16)
                nc.vector.tensor_tensor(out=sm2a[:], in0=s2a_all[:],
                                        in1=cm_sb[:], op=mybir.AluOpType.mult)
                sma = pp.tile([P, NT * K], F16)
                nc.vector.tensor_reduce(
                    out=sma[:],
                    in_=sm2a[:].rearrange("p (tk h) -> p tk h", h=2),
                    axis=mybir.AxisListType.X, op=mybir.AluOpType.add)
                ea = pp.tile([P, NT * K], F16)
                nc.scalar.activation(out=ea[:], in_=sma[:],
                                     func=mybir.ActivationFunctionType.Exp)
                sea = pp.tile([P, NT], F16)
                nc.vector.tensor_reduce(
                    out=sea[:], in_=ea[:].rearrange("p (t k) -> p t k", k=K),
                    axis=mybir.AxisListType.X, op=mybir.AluOpType.add)
                ra = pp.tile([P, NT], F16)
                nc.vector.reciprocal(out=ra[:], in_=sea[:])
                wpr = pp.tile([P, NT * K * 3], F16)
                nc.vector.tensor_tensor(
                    out=wpr[:].rearrange("p (tk e) -> p tk e", e=3),
                    in0=wsel_all[:].rearrange("p (tk e) -> p tk e", e=3),
                    in1=ea[:].unsqueeze(2).broadcast_to([P, NT * K, 3]),
                    op=mybir.AluOpType.mult)
                wsum = pp.tile([P, NT * 3], F16)
                nc.vector.tensor_reduce(
                    out=wsum[:].rearrange("p (t e) -> p t e", e=3),
                    in_=wpr[:].rearrange("p (t k e) -> p t e k", k=K, e=3),
                    axis=mybir.AxisListType.X, op=mybir.AluOpType.add)
                disp = pp.tile([P, NT * 3], F32)
                nc.vector.tensor_tensor(
                    out=disp[:].rearrange("p (t e) -> p t e", e=3),
                    in0=wsum[:].rearrange("p (t e) -> p t e", e=3),
                    in1=ra[:].unsqueeze(2).broadcast_to([P, NT, 3]),
                    op=mybir.AluOpType.mult)
                nc.vector.tensor_tensor(
                    out=out_sb[:], in0=disp[:], in1=base_sb[:],
                    op=mybir.AluOpType.add)

            nc.sync.dma_start(out=out[:, :], in_=out_sb[:])

    nc.finalize()
    return nc


def prep_inputs(sampled_points, sampled_x, edge_index_filtered,
                W_concat, b_concat, W_out, b_out, W_q, b_q, W_k, b_k,
                n_total, shard, n_tiles):
    """Host-side layout prep + weight folding. Returns in_maps for 8 cores."""
    P = 128
    valid = n_total // N_CORES
    scale = np.sqrt(np.float32(H // 2), dtype=np.float32) + 1e-6

    Wc = W_concat.astype(np.float64)
    Wo = W_out.astype(np.float64)
    M1 = Wo @ Wc[:, :H]                                    # [3,128]
    M2 = Wo @ Wc[:, H:]                                    # [3,3]
    c0 = b_concat.astype(np.float64) @ Wo.T + b_out.astype(np.float64)

    w1 = np.zeros((P, W1C), np.float64)
    w1[:, 0:64] = W_k.astype(np.float64).T
    w1[:, 64:67] = M1.T
    w1[:, 67:W1C] = W_q.astype(np.float64).T / scale
    brow = np.zeros((1, W1C), np.float64)
    # b_k is intentionally NOT folded into the table (see kernel notes)
    brow[0, 67:W1C] = b_q.astype(np.float64) / scale

    w1 = w1.astype(np.float16)
    brow_rep = np.repeat(brow.astype(np.float16), P, 0)
    c0_rep = np.repeat(c0[None].astype(np.float32), P, 0)
    m2_rep = np.repeat(M2.reshape(1, 9).astype(np.float32), P, 0)

    dst = np.asarray(edge_index_filtered[1]).reshape(n_total, K)

    # table row position of global node g, in the (core, partition, tile)
    # storage order the contiguous table write produces
    def row_pos(g):
        r = g // valid
        l = g % valid
        return r * shard + (l % P) * n_tiles + l // P

    in_maps = []
    for r in range(N_CORES):
        rows = slice(r * valid, (r + 1) * valid)
        x_r = np.zeros((shard, H), np.float16)
        x_r[:valid] = sampled_x[rows].astype(np.float16)
        nb_r = np.zeros((shard, K), np.int64)
        nb_r[:valid] = dst[rows]
        pt_r = np.zeros((shard, 3), np.float32)
        pt_r[:valid] = sampled_points[rows].astype(np.float32)

        def swz(a, width):
            return (a.reshape(n_tiles, P, width).transpose(1, 0, 2)
                    .reshape(P, n_tiles * width).copy())

        # gather indices: position (slot = t*K+k, p) -> idx = row_pos(nbr)//2,
        # stored int16 wrapped-16: [16, pos//16] replicated to all 8 partition
        # groups
        nbs = nb_r.reshape(n_tiles, P, K)
        npos = n_tiles * K * P
        pos = np.arange(npos)
        slot, p = pos // P, pos % P
        t_, k_ = slot // K, slot % K
        stream = nbs[t_, p, k_]
        srow = row_pos(stream)
        idxw = (srow // 2).astype(np.int16).reshape(-1, 16).T   # [16, npos/16]
        idx_rep = np.tile(idxw, (8, 1))                         # [128, npos/16]

        par = (srow % 2).astype(np.float32)                     # h=1 half
        nz = (stream != 0).astype(np.float32)
        # masks laid out [p, (t k h)]
        pmask = np.zeros((P, n_tiles * K * 2), np.float32)
        pmask[p, (t_ * K + k_) * 2 + 0] = 1.0 - par
        pmask[p, (t_ * K + k_) * 2 + 1] = par
        cmask = pmask.copy()
        cmask[p, (t_ * K + k_) * 2 + 0] *= nz
        cmask[p, (t_ * K + k_) * 2 + 1] *= nz

        in_maps.append({
            "ax": np.ascontiguousarray(x_r.T),
            "pts": swz(pt_r, 3),
            "idx16": np.ascontiguousarray(idx_rep),
            "cmask": cmask.astype(np.float16),
            "pmask": pmask.astype(np.float16),
            "w1": w1,
            "brow": brow_rep,
            "c0r": c0_rep,
            "m2r": m2_rep,
        })
    return in_maps


def assemble_output(results, n_total, n_tiles):
    P = 128
    valid = n_total // N_CORES
    outs = []
    for r in range(N_CORES):
        o = results[r]["out"]
        o = (o.reshape(P, n_tiles, 3).transpose(1, 0, 2)
             .reshape(n_tiles * P, 3)[:valid])
        outs.append(o)
    return np.concatenate(outs, axis=0).astype(np.float32)


_CACHED = {}


def _get_program(n_total, shard, n_tiles):
    key = (n_total, shard, n_tiles)
    if key not in _CACHED:
        _CACHED[key] = build_program(n_total, shard, n_tiles)
    return _CACHED[key]


def kernel(sampled_points, sampled_x, edge_index_filtered,
           W_concat, b_concat, W_out, b_out, W_q, b_q, W_k, b_k):
    n_total = 60000
    n_tiles = 59
    shard = n_tiles * 128
    nc = _get_program(n_total, shard, n_tiles)
    in_maps = prep_inputs(
        np.asarray(sampled_points), np.asarray(sampled_x),
        np.asarray(edge_index_filtered),
        np.asarray(W_concat), np.asarray(b_concat),
        np.asarray(W_out), np.asarray(b_out),
        np.asarray(W_q), np.asarray(b_q),
        np.asarray(W_k), np.asarray(b_k),
        n_total, shard, n_tiles)
    res = run_bass_kernel_spmd(nc, in_maps, list(range(N_CORES)))
    return assemble_output(res.results, n_total, n_tiles)


# revision 24
# speedup vs baseline: 1.1219x; 1.1219x over previous
"""Trainium2 Bass kernel for MeshGNN message passing (8 NeuronCores, SPMD).

Math reformulation (exact): since softmax weights sum to 1 and the output MLP is
linear, fold W_concat/W_out into per-node quantities:
    M1 = W_out @ W_concat[:, :128]   [3,128]
    M2 = W_out @ W_concat[:, 128:]   [3,3]
    c0 = b_concat @ W_out.T + b_out  [3]
    kx[j] = x[j] @ W_k.T + b_k                  (64,)   -> table
    w[j]  = x[j] @ M1.T + p[j] @ M2.T           (3,)    -> table
    q[n]  = (x[n] @ W_q.T + b_q) / scale        (64,)
    scores[n,k] = q[n] . kx[nbr]
    e = exp(scores * (nbr != 0))                         (scores bounded ~+-3)
    out[n] = p[n] + (-v[n] + c0) + sum_k e_k * w[nbr] / sum_k e_k,  v = p @ M2.T

Implementation notes (v5):
- Table rows are 128B: kx in fp8 e4m3 (64B) + w in fp16 (6B) + pad. The
  gather phase is DMA-transfer-bound (~35ns per random 512B descriptor on
  HW), so halving the row size halves the dominant cost. b_k is dropped from
  kx (softmax shift-invariance makes it a no-op except for the ~0.02%
  padding-masked edges, ~3e-4 rel err).
- Rows are stored in DRAM in (core, partition, tile) order so the SBUF->DRAM
  staging write is contiguous per partition (128 big descriptors instead of
  7552 strided ones, which cost ~90us of HWDGE desc-gen in v1).
- Rows are fetched with dma_gather in PAIRS (256B, idx = row//2 fits int16)
  and the correct half is selected arithmetically via host-prepared parity
  masks. Host computes pair/parity from the permuted row position.
- dma_gather desc-gen runs ~30us per 3840-idx chunk on a Q7 cpu pair, but the
  4 SWDGE queues desc-gen CONCURRENTLY when gathers are issued non-prepared
  (prepare_only blocks the Pool engine per prep and serializes the cluster,
  measured 3.4x slower end-to-end - do not use it here).
- Per-chunk DVE work is only 4 ops (score products + reduce, w select +
  reduce); softmax/value epilogue is batched once over all 59 tiles.
"""

import sys

import numpy as np

sys.path.insert(0, "/opt/trn_rl_repo")

import concourse.bass as bass
import concourse.mybir as mybir
import concourse.tile as tile
from concourse import bacc
from concourse.bass import ds, ts
from concourse.bass_utils import run_bass_kernel_spmd

N_CORES = 8
H = 128
K = 15
DT = mybir.dt
F16 = DT.float16
F32 = DT.float32
I16 = DT.int16

F8 = DT.float8e4

ROW = 128                 # fp8 elems per table row (128B): kx f8[64]|w f16[3]|pad
PAIR = 2 * ROW            # gather element: two rows (256B)
QC = 64                   # q/k dim
W1C = 131                 # matmul cols: kx(64)|w(3)|q(64)
NQ = 4                    # SWDGE queues
GATHER_BUFS = 8


def build_program(n_total, shard, n_tiles, chunk_tiles=2):
    P = 128
    nc = bacc.Bacc(None, debug=False, num_swdge_queues=NQ)

    ax = nc.declare_dram_parameter("ax", [P, shard], F16, isOutput=False)    # x.T
    pts = nc.declare_dram_parameter("pts", [P, n_tiles * 3], F32, isOutput=False)
    idx16 = nc.declare_dram_parameter("idx16", [P, n_tiles * K * 8], I16,
                                      isOutput=False)
    cmask = nc.declare_dram_parameter("cmask", [P, n_tiles * K * 2], F16,
                                      isOutput=False)
    pmask = nc.declare_dram_parameter("pmask", [P, n_tiles * K * 2], F16,
                                      isOutput=False)
    w1 = nc.declare_dram_parameter("w1", [P, W1C], F16, isOutput=False)
    brow = nc.declare_dram_parameter("brow", [P, W1C], F16, isOutput=False)
    c0r = nc.declare_dram_parameter("c0r", [P, 3], F32, isOutput=False)
    m2r = nc.declare_dram_parameter("m2r", [P, 9], F32, isOutput=False)
    out = nc.declare_dram_parameter("out", [P, n_tiles * 3], F32, isOutput=True)

    NT = n_tiles
    with tile.TileContext(nc) as tc:
        with (
            tc.tile_pool(name="persist", bufs=1) as pp,
            tc.tile_pool(name="dram", bufs=1, space="DRAM") as dp,
            tc.tile_pool(name="psum", bufs=4, space="PSUM") as psp,
            tc.tile_pool(name="kxgp", bufs=GATHER_BUFS) as kxgp,
            # bufs=2 so the ACT fp8->fp16 convert of chunk i+1 overlaps the
            # DVE score math of chunk i (bufs=1 ping-pongs the chain serially)
            tc.tile_pool(name="work", bufs=2) as wp,
        ):
            # ---- persistent SBUF ----
            xT = pp.tile([P, shard], F16)
            pts_sb = pp.tile([P, NT * 3], F32)
            idx_sb = pp.tile([P, NT * K * 8], I16)
            cm_sb = pp.tile([P, NT * K * 2], F16)
            pm_sb = pp.tile([P, NT * K * 2], F16)
            w1_sb = pp.tile([P, W1C], F16)
            br_sb = pp.tile([P, W1C], F16)
            c0_sb = pp.tile([P, 3], F32)
            m2_sb = pp.tile([P, 9], F32)
            q_sb = pp.tile([P, NT * QC], F16)
            base_sb = pp.tile([P, NT * 3], F32)
            out_sb = pp.tile([P, NT * 3], F32)
            tblall_sb = pp.tile([P, NT * ROW], F8)
            s2a_all = pp.tile([P, NT * K * 2], F16)
            wsel_all = pp.tile([P, NT * K * 3], F16)

            table_pad = dp.tile([shard, ROW], F8, space="DRAM")
            table_full = dp.tile([N_CORES * shard, ROW], F8, space="DRAM",
                                 addr_space="Shared")


            # xT + weights first: phase 1 (and thus the collective) starts as
            # early as possible; masks/indices are not needed until the gather
            # phase (~120us in)
            nc.sync.dma_start(out=w1_sb[:], in_=w1[:, :])
            nc.sync.dma_start(out=xT[:], in_=ax[:, :])
            nc.sync.dma_start(out=pts_sb[:], in_=pts[:, :])
            nc.sync.dma_start(out=br_sb[:], in_=brow[:, :])
            nc.sync.dma_start(out=c0_sb[:], in_=c0r[:, :])
            nc.sync.dma_start(out=m2_sb[:], in_=m2r[:, :])
            nc.sync.dma_start(out=idx_sb[:], in_=idx16[:, :])
            nc.sync.dma_start(out=cm_sb[:], in_=cmask[:, :])
            nc.sync.dma_start(out=pm_sb[:], in_=pmask[:, :])

            table_pairs = table_full[:].rearrange("(a two) e -> a (two e)", two=2)
            # small first chunks: the gather-phase ramp serializes the first
            # ~3 desc-gens on the Q7 cluster, so make them cheap
            chunks = []
            t0 = 0
            for _ in range(4):
                chunks.append((t0, 1))
                t0 += 1
            while t0 < n_tiles:
                chunks.append((t0, min(chunk_tiles, n_tiles - t0)))
                t0 += chunk_tiles

            def emit_gather(ci):
                t0c, nt = chunks[ci]
                nidx = nt * K * P
                kxg = kxgp.tile([P, chunk_tiles * K * PAIR], F8, tag="kxg")
                nc.gpsimd.dma_gather(
                    kxg[:, 0:nt * K * PAIR].rearrange("p (s e) -> p s e", e=PAIR),
                    table_pairs,
                    idx_sb[:, ds(t0c * K * 8, nt * K * 8)],
                    nidx, nidx, PAIR,
                    single_packet=False,
                    queue_num=ci % NQ,
                )
                return kxg

            # ---- phase 1: matmuls -> PSUM -> (table rows, q) via ACT ----
            # table row (128B): kx fp8 at byte [0:64), w fp16 at bytes [64:70).
            # b_k is dropped from kx: q.b_k is constant per node, and softmax
            # is shift-invariant (the ~0.02% padding-masked edges see a tiny
            # shift error ~3e-4 rel, well inside budget).
            tbl3 = tblall_sb[:].rearrange("p (t e) -> p t e", e=ROW)
            tblw = tbl3[:, :, 64:70].bitcast(F16)           # [P, NT, 3] f16
            q3 = q_sb[:].rearrange("p (t e) -> p t e", e=QC)
            t0 = 0
            while t0 < n_tiles:
                g = min(3, n_tiles - t0)
                ps = psp.tile([P, 3 * W1C], F32, space="PSUM", tag="ps")
                for j in range(g):
                    nc.tensor.matmul(out=ps[:, ts(j, W1C)],
                                     lhsT=xT[:, ts(t0 + j, P)], rhs=w1_sb[:],
                                     start=True, stop=True)
                ps3 = ps[:, 0:g * W1C].rearrange("p (t c) -> p t c", c=W1C)
                nc.scalar.copy(out=tbl3[:, t0:t0 + g, 0:64],
                               in_=ps3[:, :, 0:64])
                nc.scalar.copy(out=tblw[:, t0:t0 + g, :], in_=ps3[:, :, 64:67])
                nc.scalar.copy(out=q3[:, t0:t0 + g, :], in_=ps3[:, :, 67:W1C])
                t0 += 3

            # v = p @ M2.T for all tiles: [P, NT, 3]
            vp_all = pp.tile([P, NT * 9], F32)
            for j in range(3):
                nc.vector.tensor_tensor(
                    out=vp_all[:].rearrange("p (t j i) -> p t j i", j=3, i=3)
                        [:, :, j, :],
                    in0=pts_sb[:].rearrange("p (t i) -> p t i", i=3),
                    in1=m2_sb[:, ds(3 * j, 3)].unsqueeze(1)
                        .broadcast_to([P, NT, 3]),
                    op=mybir.AluOpType.mult)
            v_all = pp.tile([P, NT * 3], F32)
            nc.vector.tensor_reduce(
                out=v_all[:],
                in_=vp_all[:].rearrange("p (t j i) -> p (t j) i", j=3, i=3),
                axis=mybir.AxisListType.X, op=mybir.AluOpType.add)
            v16 = pp.tile([P, NT * 3], F16)
            nc.vector.tensor_copy(out=v16[:], in_=v_all[:])
            nc.vector.tensor_tensor(
                out=tblw, in0=tblw,
                in1=v16[:].rearrange("p (t i) -> p t i", i=3),
                op=mybir.AluOpType.add)
            nc.vector.tensor_tensor(
                out=q3, in0=q3,
                in1=br_sb[:, 67:W1C].unsqueeze(1).broadcast_to([P, NT, QC]),
                op=mybir.AluOpType.add)
            b1_all = pp.tile([P, NT * 3], F32)
            nc.vector.tensor_tensor(
                out=b1_all[:].rearrange("p (t i) -> p t i", i=3),
                in0=c0_sb[:].unsqueeze(1).broadcast_to([P, NT, 3]),
                in1=v_all[:].rearrange("p (t i) -> p t i", i=3),
                op=mybir.AluOpType.subtract)
            nc.vector.tensor_tensor(
                out=base_sb[:], in0=b1_all[:], in1=pts_sb[:],
                op=mybir.AluOpType.add)

            # ---- contiguous table write: row order (p, t) per core ----
            nc.sync.dma_start(
                out=table_pad[:].rearrange("(p t) e -> p (t e)", p=P),
                in_=tblall_sb[:])

            # ---- all-gather the fp16 table ----
            nc.gpsimd.collective_compute(
                "AllGather",
                mybir.AluOpType.bypass,
                replica_groups=[list(range(N_CORES))],
                ins=[table_pad[:].opt()],
                outs=[table_full[:].opt()],
            )

            # ---- per-chunk: trigger gather + score/value partials ----
            def emit_compute(ci, kxg):
                t0c, nt = chunks[ci]
                kx4 = kxg[:, 0:nt * K * PAIR].rearrange(
                    "p (t s e) -> p t s e", s=2 * K, e=ROW)
                qc_ap = (q_sb[:, ds(t0c * QC, nt * QC)]
                         .rearrange("p (t e) -> p t e", e=QC)
                         .unsqueeze(2).broadcast_to([P, nt, 2 * K, QC]))
                prod = wp.tile([P, chunk_tiles * K * 2 * QC], F16, tag="prod")
                pr5 = prod[:, 0:nt * K * 2 * QC].rearrange(
                    "p (t s e) -> p t s e", s=2 * K, e=QC)
                # DVE reads the fp8 kx directly (converts inline)
                nc.vector.tensor_tensor(out=pr5, in0=kx4[:, :, :, 0:QC],
                                        in1=qc_ap, op=mybir.AluOpType.mult)
                nc.vector.tensor_reduce(
                    out=s2a_all[:, ds(t0c * K * 2, nt * K * 2)], in_=pr5,
                    axis=mybir.AxisListType.X, op=mybir.AluOpType.add)
                wstage = wp.tile([P, chunk_tiles * K * 2 * 3], F16, tag="wst")
                ws4 = wstage[:, 0:nt * K * 2 * 3].rearrange(
                    "p (t s e) -> p t s e", s=2 * K, e=3)
                nc.vector.tensor_tensor(
                    out=ws4, in0=kx4[:, :, :, 64:70].bitcast(F16),
                    in1=pm_sb[:, ds(t0c * K * 2, nt * K * 2)]
                        .rearrange("p (t s) -> p t s", s=2 * K)
                        .unsqueeze(3).broadcast_to([P, nt, 2 * K, 3]),
                    op=mybir.AluOpType.mult)
                nc.vector.tensor_reduce(
                    out=wsel_all[:, ds(t0c * K * 3, nt * K * 3)]
                        .rearrange("p (tk e) -> p tk e", e=3),
                    in_=wstage[:, 0:nt * K * 2 * 3].rearrange(
                        "p (tk h e) -> p tk e h", h=2, e=3),
                    axis=mybir.AxisListType.X, op=mybir.AluOpType.add)

            with nc.allow_low_precision(
                    reason="fp16 attention intermediates; rel-err budget 2e-2"):
                for ci in range(len(chunks)):
                    emit_compute(ci, emit_gather(ci))

                # ---- batched softmax / value epilogue over all tiles ----
                sm2a = pp.tile([P, NT * K * 2], F16)
                nc.vector.tensor_tensor(out=sm2a[:], in0=s2a_all[:],
                                        in1=cm_sb[:], op=mybir.AluOpType.mult)
                sma = pp.tile([P, NT * K], F16)
                nc.vector.tensor_reduce(
                    out=sma[:],
                    in_=sm2a[:].rearrange("p (tk h) -> p tk h", h=2),
                    axis=mybir.AxisListType.X, op=mybir.AluOpType.add)
                ea = pp.tile([P, NT * K], F16)
                nc.scalar.activation(out=ea[:], in_=sma[:],
                                     func=mybir.ActivationFunctionType.Exp)
                sea = pp.tile([P, NT], F16)
                nc.vector.tensor_reduce(
                    out=sea[:], in_=ea[:].rearrange("p (t k) -> p t k", k=K),
                    axis=mybir.AxisListType.X, op=mybir.AluOpType.add)
                ra = pp.tile([P, NT], F16)
                nc.vector.reciprocal(out=ra[:], in_=sea[:])
                wpr = pp.tile([P, NT * K * 3], F16)
                nc.vector.tensor_tensor(
                    out=wpr[:].rearrange("p (tk e) -> p tk e", e=3),
                    in0=wsel_all[:].rearrange("p (tk e) -> p tk e", e=3),
                    in1=ea[:].unsqueeze(2).broadcast_to([P, NT * K, 3]),
                    op=mybir.AluOpType.mult)
                wsum = pp.tile([P, NT * 3], F16)
                nc.vector.tensor_reduce(
                    out=wsum[:].rearrange("p (t e) -> p t e", e=3),
                    in_=wpr[:].rearrange("p (t k e) -> p t e k", k=K, e=3),
                    axis=mybir.AxisListType.X, op=mybir.AluOpType.add)
                disp = pp.tile([P, NT * 3], F32)
                nc.vector.tensor_tensor(
                    out=disp[:].rearrange("p (t e) -> p t e", e=3),
                    in0=wsum[:].rearrange("p (t e) -> p t e", e=3),
                    in1=ra[:].unsqueeze(2).broadcast_to([P, NT, 3]),
                    op=mybir.AluOpType.mult)
                nc.vector.tensor_tensor(
                    out=out_sb[:], in0=disp[:], in1=base_sb[:],
                    op=mybir.AluOpType.add)

            nc.sync.dma_start(out=out[:, :], in_=out_sb[:])

    nc.finalize()
    return nc


def prep_inputs(sampled_points, sampled_x, edge_index_filtered,
                W_concat, b_concat, W_out, b_out, W_q, b_q, W_k, b_k,
                n_total, shard, n_tiles):
    """Host-side layout prep + weight folding. Returns in_maps for 8 cores."""
    P = 128
    valid = n_total // N_CORES
    scale = np.sqrt(np.float32(H // 2), dtype=np.float32) + 1e-6

    Wc = W_concat.astype(np.float64)
    Wo = W_out.astype(np.float64)
    M1 = Wo @ Wc[:, :H]                                    # [3,128]
    M2 = Wo @ Wc[:, H:]                                    # [3,3]
    c0 = b_concat.astype(np.float64) @ Wo.T + b_out.astype(np.float64)

    w1 = np.zeros((P, W1C), np.float64)
    w1[:, 0:64] = W_k.astype(np.float64).T
    w1[:, 64:67] = M1.T
    w1[:, 67:W1C] = W_q.astype(np.float64).T / scale
    brow = np.zeros((1, W1C), np.float64)
    # b_k is intentionally NOT folded into the table (see kernel notes)
    brow[0, 67:W1C] = b_q.astype(np.float64) / scale

    w1 = w1.astype(np.float16)
    brow_rep = np.repeat(brow.astype(np.float16), P, 0)
    c0_rep = np.repeat(c0[None].astype(np.float32), P, 0)
    m2_rep = np.repeat(M2.reshape(1, 9).astype(np.float32), P, 0)

    dst = np.asarray(edge_index_filtered[1]).reshape(n_total, K)

    # table row position of global node g, in the (core, partition, tile)
    # storage order the contiguous table write produces
    def row_pos(g):
        r = g // valid
        l = g % valid
        return r * shard + (l % P) * n_tiles + l // P

    in_maps = []
    for r in range(N_CORES):
        rows = slice(r * valid, (r + 1) * valid)
        x_r = np.zeros((shard, H), np.float16)
        x_r[:valid] = sampled_x[rows].astype(np.float16)
        nb_r = np.zeros((shard, K), np.int64)
        nb_r[:valid] = dst[rows]
        pt_r = np.zeros((shard, 3), np.float32)
        pt_r[:valid] = sampled_points[rows].astype(np.float32)

        def swz(a, width):
            return (a.reshape(n_tiles, P, width).transpose(1, 0, 2)
                    .reshape(P, n_tiles * width).copy())

        # gather indices: position (slot = t*K+k, p) -> idx = row_pos(nbr)//2,
        # stored int16 wrapped-16: [16, pos//16] replicated to all 8 partition
        # groups
        nbs = nb_r.reshape(n_tiles, P, K)
        npos = n_tiles * K * P
        pos = np.arange(npos)
        slot, p = pos // P, pos % P
        t_, k_ = slot // K, slot % K
        stream = nbs[t_, p, k_]
        srow = row_pos(stream)
        idxw = (srow // 2).astype(np.int16).reshape(-1, 16).T   # [16, npos/16]
        idx_rep = np.tile(idxw, (8, 1))                         # [128, npos/16]

        par = (srow % 2).astype(np.float32)                     # h=1 half
        nz = (stream != 0).astype(np.float32)
        # masks laid out [p, (t k h)]
        pmask = np.zeros((P, n_tiles * K * 2), np.float32)
        pmask[p, (t_ * K + k_) * 2 + 0] = 1.0 - par
        pmask[p, (t_ * K + k_) * 2 + 1] = par
        cmask = pmask.copy()
        cmask[p, (t_ * K + k_) * 2 + 0] *= nz
        cmask[p, (t_ * K + k_) * 2 + 1] *= nz

        in_maps.append({
            "ax": np.ascontiguousarray(x_r.T),
            "pts": swz(pt_r, 3),
            "idx16": np.ascontiguousarray(idx_rep),
            "cmask": cmask.astype(np.float16),
            "pmask": pmask.astype(np.float16),
            "w1": w1,
            "brow": brow_rep,
            "c0r": c0_rep,
            "m2r": m2_rep,
        })
    return in_maps


def assemble_output(results, n_total, n_tiles):
    P = 128
    valid = n_total // N_CORES
    outs = []
    for r in range(N_CORES):
        o = results[r]["out"]
        o = (o.reshape(P, n_tiles, 3).transpose(1, 0, 2)
             .reshape(n_tiles * P, 3)[:valid])
        outs.append(o)
    return np.concatenate(outs, axis=0).astype(np.float32)


_CACHED = {}


def _get_program(n_total, shard, n_tiles):
    key = (n_total, shard, n_tiles)
    if key not in _CACHED:
        _CACHED[key] = build_program(n_total, shard, n_tiles)
    return _CACHED[key]


def kernel(sampled_points, sampled_x, edge_index_filtered,
           W_concat, b_concat, W_out, b_out, W_q, b_q, W_k, b_k):
    n_total = 60000
    n_tiles = 59
    shard = n_tiles * 128
    nc = _get_program(n_total, shard, n_tiles)
    in_maps = prep_inputs(
        np.asarray(sampled_points), np.asarray(sampled_x),
        np.asarray(edge_index_filtered),
        np.asarray(W_concat), np.asarray(b_concat),
        np.asarray(W_out), np.asarray(b_out),
        np.asarray(W_q), np.asarray(b_q),
        np.asarray(W_k), np.asarray(b_k),
        n_total, shard, n_tiles)
    res = run_bass_kernel_spmd(nc, in_maps, list(range(N_CORES)))
    return assemble_output(res.results, n_total, n_tiles)
